# revision 8
# baseline (speedup 1.0000x reference)
"""Trainium2 Bass kernel for nn_CNN_9818295238933 (gnn_message_passing).

Data-parallel over batch: each of 8 cores owns 8 samples. Per core:
  conv1 (PE bf16) -> h1 table in SBUF, bf16 "pair" layout:
      tab[16*s + k2, f, par] = h1[sample s][k = par*16 + k2, f]
    (built directly by accumulating two half-zero weight matmuls per
     sample-pair so outputs land at 32-aligned PSUM positions)
  gather (GPSIMD ap_gather, d=2 bf16): all 8 samples in one instruction
      via per-16-partition-group index lists
  conv2 (PE, per-(n,par) blockdiag bf16, jp-split psum) -> h2 table
  gather -> conv3 -> h3 bf16 -> bounce DRAM
  AllToAll redistributes h3 so each core owns a KL=4 k-slice of fc1's
  contraction; fc1 partials accumulate in PSUM; AllReduce of y1;
  BN+ReLU+fc2+BN+ReLU+fco replicated on every core.

Self-contained: hardcodes all shapes; only imports the Trainium toolchain.
"""

import sys
from dataclasses import dataclass

if "/opt/trn_rl_repo" not in sys.path:
    sys.path.insert(0, "/opt/trn_rl_repo")

import numpy as np


@dataclass(frozen=True)
class Cfg:
    ncores: int = 8
    B: int = 64
    C: int = 12
    N: int = 7
    K: int = 32
    F: int = 9000
    FP: int = 9216
    CH: int = 512           # f-chunk per gather/conv step
    H1: int = 100
    H2: int = 30
    NCLS: int = 2
    EPS: float = 1e-5

    @property
    def BL(self):
        return self.B // self.ncores      # 8 samples per core

    @property
    def CN(self):
        return self.C * self.N            # 84

    @property
    def NCH(self):
        return self.FP // self.CH         # 18 chunks

    @property
    def KL(self):
        return self.K // self.ncores      # 4 fc1 k-rows per core

    @property
    def NI(self):
        return self.CH * self.N           # 3584 gather indices per chunk

    @property
    def K2(self):
        return self.K // 2                # 16 partition rows per sample


CFG = Cfg()


def _bf16_dtype():
    import concourse.mybir as mybir
    return mybir.dt.np(mybir.dt.bfloat16)


def _to_bf16(a):
    return np.asarray(a, dtype=np.float32).astype(_bf16_dtype())


# ---------------------------------------------------------------------------
# Host-side input preparation
# ---------------------------------------------------------------------------

def prep_core_inputs(cfg: Cfg, x, adjacencies, W1, W2, W3, fc1_w, fc1_b, bn1_g,
                     bn1_b, fc2_w, fc2_b, bn2_g, bn2_b, fco_w, fco_b):
    """Build the per-core input maps (list of dicts, one per core)."""
    B, C, N, K, F, FP, CH = (cfg.B, cfg.C, cfg.N, cfg.K, cfg.F, cfg.FP, cfg.CH)
    BL, CN, NCH, KL, NI, K2 = cfg.BL, cfg.CN, cfg.NCH, cfg.KL, cfg.NI, cfg.K2
    H1, H2, NCLS = cfg.H1, cfg.H2, cfg.NCLS

    x = np.asarray(x, dtype=np.float32)
    adj = np.asarray(adjacencies).astype(np.int64)[:, 0]  # [B, F, N]

    # x [B, C, F, N] -> per-core xt [NCH, CN, BL*CH] bf16: chunk t holds
    # all BL samples' f-slice side by side (sample-major along free).
    xt_full = np.zeros((B, CN, FP), dtype=np.float32)
    xt_full[:, :, :F] = np.transpose(x, (0, 1, 3, 2)).reshape(B, CN, F)
    xt_full = _to_bf16(xt_full)

    # Gather indices (int16 table-row indices). Chunk t, j in [0, NI):
    # j = n*CH + fl -> adj[b, t*CH + fl, n]; wrapped to
    # [j%16, t*(NI//16) + j//16] within sample s's 16-partition group.
    adjp = np.zeros((B, FP, N), dtype=np.int64)
    adjp[:, :F] = adj
    av = adjp.reshape(B, NCH, CH, N).transpose(0, 1, 3, 2).reshape(B, NCH, NI)
    aw = av.reshape(B, NCH, NI // 16, 16).transpose(0, 1, 3, 2)  # [B,NCH,16,·]

    # conv1 weights: w1f [cn, k]; pair convention k = par*16 + k2.
    # w1p[:, (par*2 + u)*32 + 16*u + k2] = w1f[:, par*16 + k2]; other cols 0.
    w1f = np.transpose(np.asarray(W1, np.float32), (1, 2, 0)).reshape(CN, K)
    w1p = np.zeros((CN, 4, 32), dtype=np.float32)
    for par in range(2):
        for u in range(2):
            w1p[:, par * 2 + u, 16 * u:16 * u + 16] = \
                w1f[:, par * 16:par * 16 + 16]
    w1p = _to_bf16(w1p.reshape(CN, 128))

    # conv2 weights: per (jp, n, par) blockdiag [128, 128]:
    #   lhsT[16s+k2, 16s+j2] = W2[jp*16+j2, par*16+k2, n]
    def blockdiag2(Wm):
        Wm = np.asarray(Wm, np.float32)
        out = np.zeros((2, N, 2, 128, 128), dtype=np.float32)
        for jp in range(2):
            for n in range(N):
                for par in range(2):
                    blk = Wm[jp * 16:jp * 16 + 16,
                             par * 16:par * 16 + 16, n].T  # [k2, j2]
                    for s in range(8):
                        out[jp, n, par, 16 * s:16 * s + 16,
                            16 * s:16 * s + 16] = blk
        return _to_bf16(out.transpose(3, 0, 1, 2, 4).reshape(128, -1))

    w2t = blockdiag2(W2)

    # conv3 weights: per (pk, n, jp) [128, 128]:
    #   lhsT[16*(pk*4+s)+j2, 32s+j'] = W3[j', jp*16+j2, n]
    W3a = np.asarray(W3, np.float32)
    wbd3 = np.zeros((2, N, 2, 128, 128), dtype=np.float32)
    for pk in range(2):
        for n in range(N):
            for jp in range(2):
                blk = W3a[:, jp * 16:jp * 16 + 16, n].T  # [j2, j']
                for s in range(4):
                    sp = pk * 4 + s
                    wbd3[pk, n, jp, 16 * sp:16 * sp + 16,
                         32 * s:32 * s + 32] = blk
    w3t = _to_bf16(wbd3.transpose(3, 0, 1, 2, 4).reshape(128, -1))

    # fc1 weights: flat r = k*F + f; core c takes k in [KL*c, KL*(c+1)).
    # Chunk t = kl*(FP//128) + f128 -> fw[p, t*H1 + h] = w[kl*FP+f128*128+p, h]
    fc1 = np.asarray(fc1_w, np.float32).reshape(H1, K, F)
    fc1t_all = np.zeros((K, FP, H1), dtype=np.float32)
    fc1t_all[:, :F] = np.transpose(fc1, (1, 2, 0))

    fc2wt = np.ascontiguousarray(np.asarray(fc2_w, np.float32).T)  # [H1, H2]
    fcowt = np.ascontiguousarray(np.asarray(fco_w, np.float32).T)  # [H2,NCLS]

    def col(v, n):
        return np.asarray(v, np.float32).reshape(n, 1)

    shared = dict(
        w1p=np.ascontiguousarray(w1p), w2t=np.ascontiguousarray(w2t),
        w3t=np.ascontiguousarray(w3t),
        fc1b=col(fc1_b, H1), bn1g=col(bn1_g, H1), bn1b=col(bn1_b, H1),
        fc2wt=fc2wt, fc2b=col(fc2_b, H2), bn2g=col(bn2_g, H2),
        bn2b=col(bn2_b, H2), fcowt=fcowt, fcob=col(fco_b, NCLS),
    )

    NCHUNK = KL * FP // 128  # 288 fc1 chunks per core

    out_maps = []
    for c in range(cfg.ncores):
        bsl = slice(c * BL, (c + 1) * BL)
        # xt [NCH, CN, BL*CH]
        xc = xt_full[bsl].reshape(BL, CN, NCH, CH)
        xt = np.ascontiguousarray(xc.transpose(2, 1, 0, 3)
                                  .reshape(NCH, CN, BL * CH))

        idx16 = np.zeros((128, NCH * (NI // 16)), dtype=np.int16)
        for s in range(BL):
            b = c * BL + s
            idx16[16 * s:16 * s + 16] = \
                aw[b].transpose(1, 0, 2).reshape(16, NCH * (NI // 16))

        blk = fc1t_all[c * KL:(c + 1) * KL].reshape(KL * FP, H1)
        fw = _to_bf16(blk.reshape(NCHUNK, 128, H1).transpose(1, 0, 2)
                      .reshape(128, NCHUNK * H1))
        m = dict(shared)
        m.update(xt=xt, idx16=np.ascontiguousarray(idx16),
                 fc1wt=np.ascontiguousarray(fw))
        out_maps.append(m)
    return out_maps


# ---------------------------------------------------------------------------
# Device program
# ---------------------------------------------------------------------------

def build_program(cfg: Cfg, reps: int = 1, probe: str = ""):
    import concourse.bass as bass
    import concourse.bacc as bacc
    import concourse.mybir as mybir
    import concourse.tile as tile
    from concourse.masks import make_identity

    f32 = mybir.dt.float32
    bf16 = mybir.dt.bfloat16
    i16 = mybir.dt.int16
    N, K, FP, CH = cfg.N, cfg.K, cfg.FP, cfg.CH
    B, BL, CN, NCH, KL, NI = cfg.B, cfg.BL, cfg.CN, cfg.NCH, cfg.KL, cfg.NI
    H1, H2, NCLS = cfg.H1, cfg.H2, cfg.NCLS
    NCORES = cfg.ncores
    NIW = NI // 16              # idx cols per chunk (224)
    NCHUNK = KL * FP // 128     # 288 fc1 chunks
    rg = [list(range(NCORES))]
    AF = mybir.ActivationFunctionType

    nc = bacc.Bacc("TRN2", target_bir_lowering=False, debug=False,
                   num_devices=NCORES, num_swdge_queues=4)

    xt = nc.dram_tensor("xt", [NCH, CN, BL * CH], bf16, kind="ExternalInput")
    idx16 = nc.dram_tensor("idx16", [128, NCH * NIW], i16,
                           kind="ExternalInput")
    w1p_d = nc.dram_tensor("w1p", [CN, 128], bf16, kind="ExternalInput")
    w2t_d = nc.dram_tensor("w2t", [128, 2 * N * 2 * 128], bf16,
                           kind="ExternalInput")
    w3t_d = nc.dram_tensor("w3t", [128, 2 * N * 2 * 128], bf16,
                           kind="ExternalInput")
    fc1wt = nc.dram_tensor("fc1wt", [128, NCHUNK * H1], bf16,
                           kind="ExternalInput")
    fc1b = nc.dram_tensor("fc1b", [H1, 1], f32, kind="ExternalInput")
    bn1g = nc.dram_tensor("bn1g", [H1, 1], f32, kind="ExternalInput")
    bn1b = nc.dram_tensor("bn1b", [H1, 1], f32, kind="ExternalInput")
    fc2wt = nc.dram_tensor("fc2wt", [H1, H2], f32, kind="ExternalInput")
    fc2b = nc.dram_tensor("fc2b", [H2, 1], f32, kind="ExternalInput")
    bn2g = nc.dram_tensor("bn2g", [H2, 1], f32, kind="ExternalInput")
    bn2b = nc.dram_tensor("bn2b", [H2, 1], f32, kind="ExternalInput")
    fcowt = nc.dram_tensor("fcowt", [H2, NCLS], f32, kind="ExternalInput")
    fcob = nc.dram_tensor("fcob", [NCLS, 1], f32, kind="ExternalInput")
    out = nc.dram_tensor("out", [NCLS, B], f32, kind="ExternalOutput")

    with tile.TileContext(nc) as tc:
        with (
            tc.tile_pool(name="consts", bufs=1) as consts,
            tc.tile_pool(name="tabs", bufs=2) as tabs,
            tc.tile_pool(name="xpool", bufs=3) as xpool,
            tc.tile_pool(name="gpool", bufs=2) as gpool,
            tc.tile_pool(name="work", bufs=2) as work,
            tc.tile_pool(name="fwpool", bufs=2) as fwpool,
            tc.tile_pool(name="dram", bufs=1, space="DRAM") as dram,
        ):
            # ---- constants ----
            ident = consts.tile([B, B], bf16)
            make_identity(nc, ident)
            zcol = consts.tile([128, 1], f32)
            nc.vector.memset(zcol[:], 0.0)
            w1_t = consts.tile([CN, 128], bf16)
            nc.sync.dma_start(w1_t[:], w1p_d[:])
            w2_t = consts.tile([128, 2 * N * 2 * 128], bf16)
            nc.sync.dma_start(w2_t[:], w2t_d[:])
            w3_t = consts.tile([128, 2 * N * 2 * 128], bf16)
            nc.sync.dma_start(w3_t[:], w3t_d[:])
            idx_t = consts.tile([128, NCH * NIW], i16)
            nc.sync.dma_start(idx_t[:], idx16[:])

            bounce = dram.tile([NCORES, KL, BL, FP], bf16)
            recv = dram.tile([NCORES, KL, BL, FP], bf16)
            y1snd = dram.tile([H1, B], f32)
            y1rcv = dram.tile([H1, B], f32)

            with tc.tile_pool(name="cpsum", bufs=1, space="PSUM") as cpsum:

                def conv1(tab):
                    """x -> h1 pair table [128, FP, 2] bf16."""
                    for t in range(NCH):
                        xs = xpool.tile([CN, BL * CH], bf16, tag="xs")
                        nc.sync.dma_start(xs[:], xt[t])
                        ps_e = cpsum.tile([128, CH], f32, tag="pa", bufs=2)
                        ps_o = cpsum.tile([128, CH], f32, tag="pb", bufs=2)
                        for q in range(4):
                            for par, ps in ((0, ps_e), (1, ps_o)):
                                for u in range(2):
                                    s = 2 * q + u
                                    nc.tensor.matmul(
                                        out=ps[32 * q:32 * q + 32, :],
                                        lhsT=w1_t[:, (par * 2 + u) * 32:
                                                  (par * 2 + u) * 32 + 32],
                                        rhs=xs[:, s * CH:(s + 1) * CH],
                                        start=(u == 0), stop=(u == 1),
                                        tile_position=(0, 32 * q))
                        dst = tab[:, t * CH:(t + 1) * CH]
                        nc.vector.tensor_copy(dst[:, :, 0], ps_e[:])
                        nc.scalar.activation(dst[:, :, 1], ps_o[:], AF.Copy)

                def glayer(src_tab, w_t, sink):
                    """Gather+conv over all chunks; sink(t, ps_a, ps_b)."""
                    for t in range(NCH):
                        g = gpool.tile([128, NI, 2], bf16, tag="g", bufs=2)
                        if probe != "nogather":
                            nc.gpsimd.ap_gather(
                                out_ap=g[:], in_ap=src_tab[:],
                                idxs_ap=idx_t[:, t * NIW:(t + 1) * NIW],
                                channels=128, num_elems=FP, d=2, num_idxs=NI)
                        ps_a = cpsum.tile([128, CH], f32, tag="pa", bufs=2)
                        ps_b = cpsum.tile([128, CH], f32, tag="pb", bufs=2)
                        if probe != "noconv":
                            st = 0
                            for n in range(N):
                                for par in range(2):
                                    rhs = g[:, n * CH:(n + 1) * CH, par]
                                    o0 = (n * 2 + par) * 128
                                    o1 = (N * 2 + n * 2 + par) * 128
                                    nc.tensor.matmul(
                                        out=ps_a[:],
                                        lhsT=w_t[:, o0:o0 + 128], rhs=rhs,
                                        start=(st == 0),
                                        stop=(st == N * 2 - 1))
                                    nc.tensor.matmul(
                                        out=ps_b[:],
                                        lhsT=w_t[:, o1:o1 + 128], rhs=rhs,
                                        start=(st == 0),
                                        stop=(st == N * 2 - 1))
                                    st += 1
                        sink(t, ps_a, ps_b)

                def conv2_sink(tab):
                    def sink(t, ps_a, ps_b):
                        dst = tab[:, t * CH:(t + 1) * CH]
                        nc.vector.tensor_copy(dst[:, :, 0], ps_a[:])
                        nc.scalar.activation(dst[:, :, 1], ps_b[:], AF.Copy)
                    return sink

                def conv3_sink():
                    # dst dims (s, d, kl, f): element order matches the
                    # SBUF source's (partition p = s*32 + d*4 + kl, f).
                    def sink(t, ps_a, ps_b):
                        h3a = work.tile([128, CH], bf16, tag="h3a", bufs=2)
                        nc.vector.tensor_copy(h3a[:], ps_a[:])
                        dsta = bounce[:, :, 0:4, t * CH:(t + 1) * CH] \
                            .transpose([2, 0, 1, 3])
                        nc.sync.dma_start(dsta, h3a[:])
                        h3b = work.tile([128, CH], bf16, tag="h3b", bufs=2)
                        nc.scalar.activation(h3b[:], ps_b[:], AF.Copy)
                        dstb = bounce[:, :, 4:8, t * CH:(t + 1) * CH] \
                            .transpose([2, 0, 1, 3])
                        nc.sync.dma_start(dstb, h3b[:])
                    return sink

                for rep in range(reps):
                    htab1 = tabs.tile([128, FP, 2], bf16, tag="tab")
                    conv1(htab1)
                    htab2 = tabs.tile([128, FP, 2], bf16, tag="tab")
                    glayer(htab1, w2_t, conv2_sink(htab2))
                    glayer(htab2, w3_t, conv3_sink())

                    tc.strict_bb_all_engine_barrier()
                    nc.gpsimd.collective_compute(
                        "AllToAll", mybir.AluOpType.bypass,
                        replica_groups=rg,
                        ins=[bounce[:].opt()], outs=[recv[:].opt()])

                    # ---- fc1 (contraction-parallel) ----
                    y1ps = cpsum.tile([H1, B], f32, tag="acc")
                    st = 0
                    for kl in range(KL):
                        fw = fwpool.tile([128, (NCHUNK // KL) * H1], bf16,
                                         tag="fw", bufs=2)
                        nc.sync.dma_start(
                            fw[:], fc1wt[:, kl * (NCHUNK // KL) * H1:
                                         (kl + 1) * (NCHUNK // KL) * H1])
                        for big in range(FP // 1024):
                            rcv = work.tile([B, 1024], bf16, tag="rcv",
                                            bufs=3)
                            src = recv[:, kl, :,
                                       big * 1024:(big + 1) * 1024]
                            nc.sync.dma_start(rcv[:], src)
                            for half in range(2):
                                tp = cpsum.tile([128, 4 * B], bf16, tag="pa",
                                                bufs=2)
                                for q in range(4):
                                    sub = half * 4 + q
                                    nc.tensor.transpose(
                                        tp[:, q * B:(q + 1) * B],
                                        rcv[:, sub * 128:(sub + 1) * 128],
                                        ident[:])
                                rT = work.tile([128, 4 * B], bf16, tag="rT",
                                               bufs=3)
                                nc.vector.tensor_copy(rT[:], tp[:])
                                for q in range(4):
                                    sub = half * 4 + q
                                    tloc = (big * 8 + sub)
                                    nc.tensor.matmul(
                                        out=y1ps[:],
                                        lhsT=fw[:, tloc * H1:(tloc + 1) * H1],
                                        rhs=rT[:, q * B:(q + 1) * B],
                                        start=(st == 0),
                                        stop=(st == NCHUNK - 1))
                                    st += 1
                    y1l = work.tile([H1, B], f32, tag="y1l")
                    nc.vector.tensor_copy(y1l[:], y1ps[:])
                    nc.sync.dma_start(y1snd[:], y1l[:])

                    nc.gpsimd.collective_compute(
                        "AllReduce", mybir.AluOpType.add, replica_groups=rg,
                        ins=[y1snd[:].opt()], outs=[y1rcv[:].opt()])

                    # ---- head (replicated on every core) ----
                    def bn_relu(y, h, g_ap, b_ap):
                        mean = work.tile([h, 1], f32, tag=f"bn_m{h}")
                        nc.vector.reduce_sum(mean[:], y[:],
                                             axis=mybir.AxisListType.X)
                        nc.vector.tensor_scalar_mul(mean[:], mean[:], 1.0 / B)
                        sq = work.tile([h, B], f32, tag=f"bn_sq{h}")
                        nc.vector.tensor_tensor(out=sq[:], in0=y[:], in1=y[:],
                                                op=mybir.AluOpType.mult)
                        var = work.tile([h, 1], f32, tag=f"bn_v{h}")
                        nc.vector.reduce_sum(var[:], sq[:],
                                             axis=mybir.AxisListType.X)
                        nc.vector.tensor_scalar_mul(var[:], var[:], 1.0 / B)
                        m2 = work.tile([h, 1], f32, tag=f"bn_m2{h}")
                        nc.vector.tensor_tensor(out=m2[:], in0=mean[:],
                                                in1=mean[:],
                                                op=mybir.AluOpType.mult)
                        nc.vector.tensor_tensor(out=var[:], in0=var[:],
                                                in1=m2[:],
                                                op=mybir.AluOpType.subtract)
                        nc.vector.tensor_scalar_add(var[:], var[:], cfg.EPS)
                        std = work.tile([h, 1], f32, tag=f"bn_s{h}")
                        nc.scalar.activation(std[:], var[:], AF.Sqrt,
                                             bias=zcol[0:h, 0:1])
                        rstd = work.tile([h, 1], f32, tag=f"bn_r{h}")
                        nc.vector.reciprocal(rstd[:], std[:])
                        gl = work.tile([h, 1], f32, tag=f"bn_g{h}")
                        nc.sync.dma_start(gl[:], g_ap[:])
                        bl = work.tile([h, 1], f32, tag=f"bn_b{h}")
                        nc.sync.dma_start(bl[:], b_ap[:])
                        scale = work.tile([h, 1], f32, tag=f"bn_sc{h}")
                        nc.vector.tensor_tensor(out=scale[:], in0=rstd[:],
                                                in1=gl[:],
                                                op=mybir.AluOpType.mult)
                        shift = work.tile([h, 1], f32, tag=f"bn_sh{h}")
                        nc.vector.tensor_tensor(out=shift[:], in0=mean[:],
                                                in1=scale[:],
                                                op=mybir.AluOpType.mult)
                        nc.vector.tensor_tensor(out=shift[:], in0=bl[:],
                                                in1=shift[:],
                                                op=mybir.AluOpType.subtract)
                        nc.vector.tensor_scalar(
                            out=y[:], in0=y[:], scalar1=scale[:],
                            scalar2=shift[:], op0=mybir.AluOpType.mult,
                            op1=mybir.AluOpType.add)
                        nc.scalar.activation(y[:], y[:], AF.Relu,
                                             bias=zcol[0:h, 0:1])

                    y1 = work.tile([H1, B], f32, tag="y1h")
                    nc.sync.dma_start(y1[:], y1rcv[:])
                    f1b = work.tile([H1, 1], f32, tag="f1b")
                    nc.sync.dma_start(f1b[:], fc1b[:])
                    nc.vector.tensor_scalar_add(y1[:], y1[:], f1b[:])
                    bn_relu(y1, H1, bn1g, bn1b)

                    w2f = work.tile([H1, H2], f32, tag="w2f")
                    nc.sync.dma_start(w2f[:], fc2wt[:])
                    ps2 = cpsum.tile([H2, B], f32, tag="pb", bufs=2)
                    nc.tensor.matmul(out=ps2[:], lhsT=w2f[:], rhs=y1[:],
                                     start=True, stop=True)
                    y2 = work.tile([H2, B], f32, tag="y2h")
                    nc.vector.tensor_copy(y2[:], ps2[:])
                    f2b = work.tile([H2, 1], f32, tag="f2b")
                    nc.sync.dma_start(f2b[:], fc2b[:])
                    nc.vector.tensor_scalar_add(y2[:], y2[:], f2b[:])
                    bn_relu(y2, H2, bn2g, bn2b)

                    wof = work.tile([H2, NCLS], f32, tag="wof")
                    nc.sync.dma_start(wof[:], fcowt[:])
                    pso = cpsum.tile([NCLS, B], f32, tag="pb", bufs=2)
                    nc.tensor.matmul(out=pso[:], lhsT=wof[:], rhs=y2[:],
                                     start=True, stop=True)
                    yo = work.tile([NCLS, B], f32, tag="yo")
                    nc.vector.tensor_copy(yo[:], pso[:])
                    fob = work.tile([NCLS, 1], f32, tag="fob")
                    nc.sync.dma_start(fob[:], fcob[:])
                    nc.vector.tensor_scalar_add(yo[:], yo[:], fob[:])
                    nc.sync.dma_start(out[:], yo[:])

    nc.compile()
    return nc


_CACHE: dict = {}


def _get_program(cfg: Cfg, reps: int = 1, probe: str = ""):
    key = (cfg, reps, probe)
    if key not in _CACHE:
        _CACHE[key] = build_program(cfg, reps=reps, probe=probe)
    return _CACHE[key]


def kernel(**inputs) -> np.ndarray:
    from concourse import bass_utils

    cfg = CFG
    nc = _get_program(cfg)
    in_maps = prep_core_inputs(cfg, **inputs)
    res = bass_utils.run_bass_kernel_spmd(
        nc, in_maps, core_ids=list(range(cfg.ncores)))
    return np.ascontiguousarray(
        res.results[0]["out"].T.astype(np.float32))
